# revision 10
# baseline (speedup 1.0000x reference)
"""Trainium2 Bass kernel for nn_Encoder_Postnet (length-regulator gather + per-frame linears).

Contract: kernel(**inputs) takes FULL numpy inputs (as produced by
setup_inputs) and returns the FULL [B, T, H] float32 output. Internally the
batch dim is sharded across 8 NeuronCores (pure data parallel, 4 batches per
core); the tiny Linear(1,H) params are replicated.

Per-core algorithm (BPC=4 batches, T=4096 frames, P=512 phonemes, H=512):
  1. idx[b,t] = cumsum_t(align[b,t] != align[b,t-1])  -- DVE compare + scan
  2. PE-transpose idx chunks into per-partition layout -> gather offsets
  3. grouped indirect-DMA gathers: enc[b, idx[b,t], :] rows from HBM
  4. rank-4 PE matmul per 128-frame tile:
       [pitch_t, beats_t, t, 1] @ [w_pitch; w_beats; w_pos; b_sum]
  5. one DVE add (gathered + psum) per tile, HWDGE write out
"""

import sys

if "/opt/trn_rl_repo" not in sys.path:
    sys.path.insert(0, "/opt/trn_rl_repo")

from contextlib import ExitStack

import numpy as np

import concourse.bass as bass
import concourse.tile as tile
from concourse import bacc, mybir
from concourse.bass_utils import run_bass_kernel_spmd
from concourse.masks import make_identity

B, T, P, H = 32, 4096, 512, 512
NCORES = 8
BPC = B // NCORES            # batches per core
TILE_T = 128                 # frames per tile (partition dim)
NCHUNK = T // TILE_T         # 32 tiles per batch
GROUP = 8                    # tiles per indirect-gather call
K_MM = 6                     # matmul contraction: iota, pitch, beats, 1, 1, 1
F32 = mybir.dt.float32
I32 = mybir.dt.int32
ADD = mybir.AluOpType.add
NE = mybir.AluOpType.not_equal


def _emit(ctx: ExitStack, tc: tile.TileContext, enc, pitch, beats, align,
          w_pitch, w_beats, w_pos, b_pitch, b_beats, b_pos, aux, out):
    nc = tc.nc
    const = ctx.enter_context(tc.tile_pool(name="const", bufs=1))
    apool = ctx.enter_context(tc.tile_pool(name="apool", bufs=2))
    gpool = ctx.enter_context(tc.tile_pool(name="gpool", bufs=3))
    ppool = ctx.enter_context(tc.tile_pool(name="ppool", bufs=4, space="PSUM"))
    tpsum = ctx.enter_context(tc.tile_pool(name="tpsum", bufs=1, space="PSUM"))

    # --- W [6, H]: rows = w_pos, w_pitch, w_beats, b_pitch, b_beats, b_pos
    # (matches A rows: iota, pitch, beats, 1, 1, 1 — K=6 matmul folds the
    # biases in, so no compute ever touches a non-quarter start partition)
    W = const.tile([K_MM, H], F32)
    nc.sync.dma_start(W[0:1, :], w_pos[None, :])
    nc.sync.dma_start(W[1:2, :], w_pitch[None, :])
    nc.sync.dma_start(W[2:3, :], w_beats[None, :])
    nc.sync.dma_start(W[3:4, :], b_pitch[None, :])
    nc.sync.dma_start(W[4:5, :], b_beats[None, :])
    nc.sync.dma_start(W[5:6, :], b_pos[None, :])

    # --- idx[b, t] = cumsum_t(align[b, t] != align[b, t-1]), in f32 (exact)
    align_sb = const.tile([BPC, T], I32)
    nc.sync.dma_start(align_sb[:], align[:])
    change = const.tile([BPC, T], F32)
    nc.vector.memset(change[:, 0:1], 0.0)
    nc.vector.tensor_tensor(change[:, 1:T], align_sb[:, 1:T],
                            align_sb[:, 0:T - 1], op=NE)
    zeros = const.tile([BPC, T], F32)
    nc.vector.memset(zeros[:], 0.0)
    idxf = const.tile([BPC, T], F32)
    nc.vector.tensor_tensor_scan(idxf[:], change[:], zeros[:], 0.0,
                                 op0=ADD, op1=ADD)

    # --- transpose to per-partition layout: idxT[p, c*BPC + b] = idx[b, c*128+p]
    ident = const.tile([BPC, BPC], F32)
    make_identity(nc, ident[:])
    idxT_ps = tpsum.tile([TILE_T, NCHUNK * BPC], F32)
    for c in range(NCHUNK):
        nc.tensor.transpose(idxT_ps[:, c * BPC:(c + 1) * BPC],
                            idxf[:, c * TILE_T:(c + 1) * TILE_T], ident[:])
    idxT = const.tile([TILE_T, NCHUNK * BPC], F32)
    nc.vector.tensor_copy(idxT[:], idxT_ps[:])

    # --- per-batch row offsets into enc viewed as [(BPC*P), H]: idx + b*P
    idxT3 = idxT[:].rearrange("p (c b) -> p b c", b=BPC)  # [128, BPC, NCHUNK]
    offs = []
    for b in range(BPC):
        ob = const.tile([TILE_T, NCHUNK], I32, tag=f"offs{b}")
        nc.vector.tensor_scalar_add(ob[:], idxT3[:, b, :], float(b * P))
        offs.append(ob)

    # --- main loop
    for b in range(BPC):
        # A [6, T]: rows = iota(T), pitch[b], beats[b], 1, 1, 1
        # (iota/ones rows come from the host-side aux constant via DMA —
        # compute ops can't start at partition 1..5)
        A = apool.tile([K_MM, T], F32)
        nc.sync.dma_start(A[0:1, :], aux[0:1, :])
        nc.sync.dma_start(A[1:2, :], pitch[b:b + 1, :])
        nc.sync.dma_start(A[2:3, :], beats[b:b + 1, :])
        nc.sync.dma_start(A[3:4, :], aux[1:2, :])
        nc.sync.dma_start(A[4:5, :], aux[1:2, :])
        nc.sync.dma_start(A[5:6, :], aux[1:2, :])

        for g in range(NCHUNK // GROUP):
            gt = gpool.tile([TILE_T, GROUP * H], F32)
            nc.gpsimd.indirect_dma_start(
                out=gt[:],
                out_offset=None,
                in_=enc[:],
                in_offset=bass.IndirectOffsetOnAxis(
                    ap=offs[b][:, g * GROUP:(g + 1) * GROUP], axis=0),
            )
            for k in range(GROUP):
                c = g * GROUP + k
                ps = ppool.tile([TILE_T, H], F32)
                nc.tensor.matmul(ps[:],
                                 lhsT=A[:, c * TILE_T:(c + 1) * TILE_T],
                                 rhs=W[:], start=True, stop=True)
                nc.vector.tensor_tensor(gt[:, k * H:(k + 1) * H],
                                        gt[:, k * H:(k + 1) * H], ps[:], op=ADD)
                nc.sync.dma_start(
                    out[b * T + c * TILE_T: b * T + (c + 1) * TILE_T, :],
                    gt[:, k * H:(k + 1) * H])


_CACHED = None


def _build():
    global _CACHED
    if _CACHED is not None:
        return _CACHED
    nc = bacc.Bacc("TRN2", target_bir_lowering=False, debug=False)
    enc = nc.dram_tensor("enc", (BPC * P, H), F32, kind="ExternalInput").ap()
    pitch = nc.dram_tensor("pitch", (BPC, T), F32, kind="ExternalInput").ap()
    beats = nc.dram_tensor("beats", (BPC, T), F32, kind="ExternalInput").ap()
    align = nc.dram_tensor("align", (BPC, T), I32, kind="ExternalInput").ap()
    w_pitch = nc.dram_tensor("w_pitch", (H,), F32, kind="ExternalInput").ap()
    w_beats = nc.dram_tensor("w_beats", (H,), F32, kind="ExternalInput").ap()
    w_pos = nc.dram_tensor("w_pos", (H,), F32, kind="ExternalInput").ap()
    b_pitch = nc.dram_tensor("b_pitch", (H,), F32, kind="ExternalInput").ap()
    b_beats = nc.dram_tensor("b_beats", (H,), F32, kind="ExternalInput").ap()
    b_pos = nc.dram_tensor("b_pos", (H,), F32, kind="ExternalInput").ap()
    aux = nc.dram_tensor("aux", (2, T), F32, kind="ExternalInput").ap()
    out = nc.dram_tensor("out", (BPC * T, H), F32, kind="ExternalOutput").ap()

    with tile.TileContext(nc) as tc:
        with ExitStack() as ctx:
            _emit(ctx, tc, enc, pitch, beats, align, w_pitch, w_beats, w_pos,
                  b_pitch, b_beats, b_pos, aux, out)
    nc.compile()
    _CACHED = nc
    return nc


def make_in_maps(encoder_out, pitch, beats, align_phone,
                 w_pitch, b_pitch, w_beats, b_beats, w_pos, b_pos):
    aux = np.stack([np.arange(T, dtype=np.float32),
                    np.ones(T, dtype=np.float32)])
    reps = {
        "aux": aux,
        "w_pitch": np.ascontiguousarray(w_pitch, np.float32),
        "w_beats": np.ascontiguousarray(w_beats, np.float32),
        "w_pos": np.ascontiguousarray(w_pos, np.float32),
        "b_pitch": np.ascontiguousarray(b_pitch, np.float32),
        "b_beats": np.ascontiguousarray(b_beats, np.float32),
        "b_pos": np.ascontiguousarray(b_pos, np.float32),
    }
    in_maps = []
    for r in range(NCORES):
        s = slice(r * BPC, (r + 1) * BPC)
        in_maps.append({
            "enc": np.ascontiguousarray(
                encoder_out[s], np.float32).reshape(BPC * P, H),
            "pitch": np.ascontiguousarray(pitch[s], np.float32),
            "beats": np.ascontiguousarray(beats[s], np.float32),
            "align": np.ascontiguousarray(align_phone[s], np.int32),
            **reps,
        })
    return in_maps


def kernel(encoder_out, pitch, beats, w_pitch, b_pitch, w_beats, b_beats,
           w_pos, b_pos, align_phone, _trace=False):
    nc = _build()
    in_maps = make_in_maps(encoder_out, pitch, beats, align_phone,
                           w_pitch, b_pitch, w_beats, b_beats, w_pos, b_pos)
    res = run_bass_kernel_spmd(nc, in_maps, core_ids=list(range(NCORES)),
                               trace=_trace)
    out = np.concatenate(
        [res.results[r]["out"].reshape(BPC, T, H) for r in range(NCORES)],
        axis=0)
    if _trace:
        kernel.last_results = res
    return out


# revision 12
# speedup vs baseline: 1.0312x; 1.0312x over previous
"""Trainium2 Bass kernel for nn_Encoder_Postnet (length-regulator gather + per-frame linears).

Contract: kernel(**inputs) takes FULL numpy inputs (as produced by
setup_inputs) and returns the FULL [B, T, H] float32 output. Internally the
batch dim is sharded across 8 NeuronCores (pure data parallel, 4 batches per
core); the tiny Linear(1,H) params are replicated.

Per-core algorithm (BPC=4 batches, T=4096 frames, P=512 phonemes, H=512):
  1. idx[b,t] = cumsum_t(align[b,t] != align[b,t-1])  -- DVE compare + scan
  2. PE-transpose idx chunks into per-partition layout -> gather offsets
  3. grouped indirect-DMA gathers: enc[b, idx[b,t], :] rows from HBM
  4. rank-4 PE matmul per 128-frame tile:
       [pitch_t, beats_t, t, 1] @ [w_pitch; w_beats; w_pos; b_sum]
  5. one DVE add (gathered + psum) per tile, HWDGE write out
"""

import sys

if "/opt/trn_rl_repo" not in sys.path:
    sys.path.insert(0, "/opt/trn_rl_repo")

from contextlib import ExitStack

import numpy as np

import concourse.bass as bass
import concourse.tile as tile
from concourse import bacc, mybir
from concourse.bass_utils import run_bass_kernel_spmd
from concourse.masks import make_identity

B, T, P, H = 32, 4096, 512, 512
NCORES = 8
BPC = B // NCORES            # batches per core
TILE_T = 128                 # frames per tile (partition dim)
NCHUNK = T // TILE_T         # 32 tiles per batch
GROUP = 8                    # tiles per indirect-gather call
K_MM = 6                     # matmul contraction: iota, pitch, beats, 1, 1, 1
F32 = mybir.dt.float32
I32 = mybir.dt.int32
ADD = mybir.AluOpType.add
NE = mybir.AluOpType.not_equal


def _emit(ctx: ExitStack, tc: tile.TileContext, enc, pitch, beats, align,
          w_pitch, w_beats, w_pos, b_pitch, b_beats, b_pos, aux, out):
    nc = tc.nc
    const = ctx.enter_context(tc.tile_pool(name="const", bufs=1))
    apool = ctx.enter_context(tc.tile_pool(name="apool", bufs=2))
    gpool = ctx.enter_context(tc.tile_pool(name="gpool", bufs=6))
    ppool = ctx.enter_context(tc.tile_pool(name="ppool", bufs=4, space="PSUM"))
    tpsum = ctx.enter_context(tc.tile_pool(name="tpsum", bufs=1, space="PSUM"))

    # --- W [6, H]: rows = w_pos, w_pitch, w_beats, b_pitch, b_beats, b_pos
    # (matches A rows: iota, pitch, beats, 1, 1, 1 — K=6 matmul folds the
    # biases in, so no compute ever touches a non-quarter start partition)
    W = const.tile([K_MM, H], F32)
    nc.sync.dma_start(W[0:1, :], w_pos[None, :])
    nc.sync.dma_start(W[1:2, :], w_pitch[None, :])
    nc.sync.dma_start(W[2:3, :], w_beats[None, :])
    nc.sync.dma_start(W[3:4, :], b_pitch[None, :])
    nc.sync.dma_start(W[4:5, :], b_beats[None, :])
    nc.sync.dma_start(W[5:6, :], b_pos[None, :])

    # --- idx[b, t] = cumsum_t(align[b, t] != align[b, t-1]), in f32 (exact)
    align_sb = const.tile([BPC, T], I32)
    nc.sync.dma_start(align_sb[:], align[:])
    change = const.tile([BPC, T], F32)
    nc.vector.memset(change[:, 0:1], 0.0)
    nc.vector.tensor_tensor(change[:, 1:T], align_sb[:, 1:T],
                            align_sb[:, 0:T - 1], op=NE)
    zeros = const.tile([BPC, T], F32)
    nc.vector.memset(zeros[:], 0.0)
    idxf = const.tile([BPC, T], F32)
    nc.vector.tensor_tensor_scan(idxf[:], change[:], zeros[:], 0.0,
                                 op0=ADD, op1=ADD)

    # --- transpose to per-partition layout: idxT[p, c*BPC + b] = idx[b, c*128+p]
    ident = const.tile([BPC, BPC], F32)
    make_identity(nc, ident[:])
    idxT_ps = tpsum.tile([TILE_T, NCHUNK * BPC], F32)
    for c in range(NCHUNK):
        nc.tensor.transpose(idxT_ps[:, c * BPC:(c + 1) * BPC],
                            idxf[:, c * TILE_T:(c + 1) * TILE_T], ident[:])
    idxT = const.tile([TILE_T, NCHUNK * BPC], F32)
    nc.vector.tensor_copy(idxT[:], idxT_ps[:])

    # --- per-batch row offsets into enc viewed as [(BPC*P), H]: idx + b*P
    idxT3 = idxT[:].rearrange("p (c b) -> p b c", b=BPC)  # [128, BPC, NCHUNK]
    offs = []
    for b in range(BPC):
        ob = const.tile([TILE_T, NCHUNK], I32, tag=f"offs{b}")
        nc.vector.tensor_scalar_add(ob[:], idxT3[:, b, :], float(b * P))
        offs.append(ob)

    # --- main loop
    for b in range(BPC):
        # A [6, T]: rows = iota(T), pitch[b], beats[b], 1, 1, 1
        # (iota/ones rows come from the host-side aux constant via DMA —
        # compute ops can't start at partition 1..5)
        A = apool.tile([K_MM, T], F32)
        nc.sync.dma_start(A[0:1, :], aux[0:1, :])
        nc.sync.dma_start(A[1:2, :], pitch[b:b + 1, :])
        nc.sync.dma_start(A[2:3, :], beats[b:b + 1, :])
        nc.sync.dma_start(A[3:4, :], aux[1:2, :])
        nc.sync.dma_start(A[4:5, :], aux[1:2, :])
        nc.sync.dma_start(A[5:6, :], aux[1:2, :])

        for c in range(NCHUNK):
            # HW indirect DMA consumes exactly one offset per dest partition,
            # so gathers are per-chunk: 128 descriptors x one H-row each
            gt = gpool.tile([TILE_T, H], F32)
            nc.gpsimd.indirect_dma_start(
                out=gt[:],
                out_offset=None,
                in_=enc[:],
                in_offset=bass.IndirectOffsetOnAxis(
                    ap=offs[b][:, c:c + 1], axis=0),
            )
            ps = ppool.tile([TILE_T, H], F32)
            nc.tensor.matmul(ps[:],
                             lhsT=A[:, c * TILE_T:(c + 1) * TILE_T],
                             rhs=W[:], start=True, stop=True)
            nc.vector.tensor_tensor(gt[:], gt[:], ps[:], op=ADD)
            nc.sync.dma_start(
                out[b * T + c * TILE_T: b * T + (c + 1) * TILE_T, :], gt[:])


_CACHED = None


def _build():
    global _CACHED
    if _CACHED is not None:
        return _CACHED
    nc = bacc.Bacc("TRN2", target_bir_lowering=False, debug=False)
    enc = nc.dram_tensor("enc", (BPC * P, H), F32, kind="ExternalInput").ap()
    pitch = nc.dram_tensor("pitch", (BPC, T), F32, kind="ExternalInput").ap()
    beats = nc.dram_tensor("beats", (BPC, T), F32, kind="ExternalInput").ap()
    align = nc.dram_tensor("align", (BPC, T), I32, kind="ExternalInput").ap()
    w_pitch = nc.dram_tensor("w_pitch", (H,), F32, kind="ExternalInput").ap()
    w_beats = nc.dram_tensor("w_beats", (H,), F32, kind="ExternalInput").ap()
    w_pos = nc.dram_tensor("w_pos", (H,), F32, kind="ExternalInput").ap()
    b_pitch = nc.dram_tensor("b_pitch", (H,), F32, kind="ExternalInput").ap()
    b_beats = nc.dram_tensor("b_beats", (H,), F32, kind="ExternalInput").ap()
    b_pos = nc.dram_tensor("b_pos", (H,), F32, kind="ExternalInput").ap()
    aux = nc.dram_tensor("aux", (2, T), F32, kind="ExternalInput").ap()
    out = nc.dram_tensor("out", (BPC * T, H), F32, kind="ExternalOutput").ap()

    with tile.TileContext(nc) as tc:
        with ExitStack() as ctx:
            _emit(ctx, tc, enc, pitch, beats, align, w_pitch, w_beats, w_pos,
                  b_pitch, b_beats, b_pos, aux, out)
    nc.compile()
    _CACHED = nc
    return nc


def make_in_maps(encoder_out, pitch, beats, align_phone,
                 w_pitch, b_pitch, w_beats, b_beats, w_pos, b_pos):
    aux = np.stack([np.arange(T, dtype=np.float32),
                    np.ones(T, dtype=np.float32)])
    reps = {
        "aux": aux,
        "w_pitch": np.ascontiguousarray(w_pitch, np.float32),
        "w_beats": np.ascontiguousarray(w_beats, np.float32),
        "w_pos": np.ascontiguousarray(w_pos, np.float32),
        "b_pitch": np.ascontiguousarray(b_pitch, np.float32),
        "b_beats": np.ascontiguousarray(b_beats, np.float32),
        "b_pos": np.ascontiguousarray(b_pos, np.float32),
    }
    in_maps = []
    for r in range(NCORES):
        s = slice(r * BPC, (r + 1) * BPC)
        in_maps.append({
            "enc": np.ascontiguousarray(
                encoder_out[s], np.float32).reshape(BPC * P, H),
            "pitch": np.ascontiguousarray(pitch[s], np.float32),
            "beats": np.ascontiguousarray(beats[s], np.float32),
            "align": np.ascontiguousarray(align_phone[s], np.int32),
            **reps,
        })
    return in_maps


def kernel(encoder_out, pitch, beats, w_pitch, b_pitch, w_beats, b_beats,
           w_pos, b_pos, align_phone, _trace=False):
    nc = _build()
    in_maps = make_in_maps(encoder_out, pitch, beats, align_phone,
                           w_pitch, b_pitch, w_beats, b_beats, w_pos, b_pos)
    res = run_bass_kernel_spmd(nc, in_maps, core_ids=list(range(NCORES)),
                               trace=_trace)
    out = np.concatenate(
        [res.results[r]["out"].reshape(BPC, T, H) for r in range(NCORES)],
        axis=0)
    if _trace:
        kernel.last_results = res
    return out
